# revision 1
# baseline (speedup 1.0000x reference)
"""Multi-head causal attention (b=4, l=2048, d=1024, 16 heads x 64) on 8 trn2 cores.

Sharding: core c handles batch (c // 2) and head-group (c % 2) of 8 heads.
Each core computes a partial output x[b] @ W (its 8 heads' contribution);
the host sums the two partials per batch.

Device layouts (per core):
  xT      [1024, 2048]   x[b] transposed on host (d on partitions)
  wq/wk   [1024, 512]    head-group column slices (natural layout, lhsT)
  wv      [1024, 512]
  wo      [512, 1024]    head-group row slice (rhs)
  qT/kT   [512, 2048]    c on partitions (4 sbuf tensors of 128)
  v_pad   16 x [128, 8, 65]  v natural (l on partitions), per head 64 cols + ones col
  S^T     [128 m, 512 q] tiles -> exp -> P^T; PV: O'^T = [V|1]^T P^T gives sums row
  softmax uses no max-subtraction (scores are O(1)); the fully-masked q=0
  column is fixed up with a uniform mean-of-V matmul.

All matmul inputs are float32r (~tf32 precision, 4x faster than fp32 on PE).
"""

import os
import sys

sys.path.insert(0, "/opt/trn_rl_repo")

import numpy as np

import concourse.bacc as bacc
import concourse.mybir as mybir
import concourse.tile as tile
from concourse.bass_utils import run_bass_kernel_spmd

F32 = mybir.dt.float32
F32R = mybir.dt.float32r
AF = mybir.ActivationFunctionType
ALU = mybir.AluOpType

B, L, D = 4, 2048, 1024
N_HEAD, KEY_DIM = 16, 64
HG = 8               # heads per core (head-group)
C = HG * KEY_DIM     # 512 per-core qkv width
SCALE = 1.0 / 8.0    # 1/sqrt(KEY_DIM)
NLC = 16             # l chunks of 128
NJ = 4               # q chunks of 512
ND = 8               # d chunks of 128
NCC = 4              # c chunks of 128

_CACHED = {}


def build_nc():
    nc = bacc.Bacc("TRN2", target_bir_lowering=False, debug=False)

    xT = nc.dram_tensor("xT", [D, L], F32R, kind="ExternalInput")
    wq = nc.dram_tensor("wq", [D, C], F32R, kind="ExternalInput")
    wk = nc.dram_tensor("wk", [D, C], F32R, kind="ExternalInput")
    wv = nc.dram_tensor("wv", [D, C], F32R, kind="ExternalInput")
    wo = nc.dram_tensor("wo", [C, D], F32R, kind="ExternalInput")
    out = nc.dram_tensor("out", [L, D], F32, kind="ExternalOutput")

    with tile.TileContext(nc) as tc:
        # ---- persistent pools (live across all phases) ----
        with tc.tile_pool(name="persist", bufs=1) as persist, \
             tc.tile_pool(name="const", bufs=1) as constp:

            qT = [persist.tile([128, L], F32R, name=f"qT{t}") for t in range(NCC)]
            kT = [persist.tile([128, L], F32R, name=f"kT{t}") for t in range(NCC)]
            vp = [persist.tile([128, HG, KEY_DIM + 1], F32R, name=f"vp{i}")
                  for i in range(NLC)]
            masks = [constp.tile([128, 512], F32, name=f"mask{r}") for r in range(4)]

            # constants
            for r in range(4):
                nc.gpsimd.memset(masks[r][:], 1.0)
                # keep where f - p - 128*r > 0 (i.e. q > m), else 0
                nc.gpsimd.affine_select(
                    out=masks[r][:], in_=masks[r][:],
                    compare_op=ALU.is_gt, fill=0.0,
                    base=-(128 * r), channel_multiplier=-1, pattern=[[1, 512]],
                )
            for i in range(NLC):
                # whole-tile memset (strided column memset fails ISA check);
                # phase-2 copies overwrite cols 0..63, col 64 stays 1.0
                nc.vector.memset(vp[i][:].bitcast(F32), 1.0)

            # ---- phase 1+2: stream xT, project q/k/v ----
            with tc.tile_pool(name="wqkv", bufs=1) as wpool, \
                 tc.tile_pool(name="xt", bufs=8) as xtp, \
                 tc.tile_pool(name="psA", bufs=3, space="PSUM") as psA:
                wq_sb = [wpool.tile([128, C], F32R, name=f"wq{d}") for d in range(ND)]
                wk_sb = [wpool.tile([128, C], F32R, name=f"wk{d}") for d in range(ND)]
                wv_sb = [wpool.tile([128, C], F32R, name=f"wv{d}") for d in range(ND)]
                for d in range(ND):
                    nc.sync.dma_start(wq_sb[d][:], wq[128 * d:128 * (d + 1), :])
                    nc.sync.dma_start(wk_sb[d][:], wk[128 * d:128 * (d + 1), :])
                    nc.sync.dma_start(wv_sb[d][:], wv[128 * d:128 * (d + 1), :])

                for lc in range(NJ):  # 4 l-chunks of 512
                    ls = slice(512 * lc, 512 * (lc + 1))
                    xts = []
                    for d in range(ND):
                        t = xtp.tile([128, 512], F32R, name=f"xt{lc}_{d}", tag="xt")
                        nc.sync.dma_start(t[:], xT[128 * d:128 * (d + 1), ls])
                        xts.append(t)
                    # qT / kT chunks: out [128 c, 512 l]
                    for w_sb, dst in ((wq_sb, qT), (wk_sb, kT)):
                        for cc in range(NCC):
                            ps = psA.tile([128, 512], F32, name=f"pqk{lc}{cc}", tag="psA")
                            for d in range(ND):
                                nc.tensor.matmul(
                                    ps[:], w_sb[d][:, 128 * cc:128 * (cc + 1)],
                                    xts[d][:], start=(d == 0), stop=(d == ND - 1))
                            nc.scalar.copy(dst[cc][:, ls], ps[:])
                    # v natural: out [128 l, 512 c] -> strided into vp
                    for lcc in range(4):
                        i = 4 * lc + lcc
                        ps = psA.tile([128, 512], F32, name=f"pv{i}", tag="psA")
                        for d in range(ND):
                            nc.tensor.matmul(
                                ps[:], xts[d][:, 128 * lcc:128 * (lcc + 1)],
                                wv_sb[d][:], start=(d == 0), stop=(d == ND - 1))
                        nc.scalar.copy(
                            vp[i][:, :, 0:KEY_DIM],
                            ps[:].rearrange("p (h c) -> p h c", h=HG))

            # ---- phase 3+4 (j-major): attention + output projection ----
            with tc.tile_pool(name="of", bufs=1) as ofp, \
                 tc.tile_pool(name="wo", bufs=1) as wop, \
                 tc.tile_pool(name="pp", bufs=10) as pp, \
                 tc.tile_pool(name="ep", bufs=3) as ep, \
                 tc.tile_pool(name="bcp", bufs=2) as bcp, \
                 tc.tile_pool(name="ovp", bufs=4) as ovp, \
                 tc.tile_pool(name="osb", bufs=3) as osb, \
                 tc.tile_pool(name="rp", bufs=2) as rp, \
                 tc.tile_pool(name="psS", bufs=4, space="PSUM") as psS, \
                 tc.tile_pool(name="psO", bufs=2, space="PSUM") as psO, \
                 tc.tile_pool(name="psF", bufs=2, space="PSUM") as psF:

                OF = [ofp.tile([128, L], F32R, name=f"of{t}") for t in range(NCC)]
                wo_sb = [wop.tile([128, D], F32R, name=f"wo{t}") for t in range(NCC)]
                for t in range(NCC):
                    nc.sync.dma_start(wo_sb[t][:], wo[128 * t:128 * (t + 1), :])

                for j in range(NJ):
                    js = slice(512 * j, 512 * (j + 1))
                    n_i = 4 * j + 4
                    for hp in range(4):  # head pairs share kT/qT tensor hp
                        # even head on partitions 0:64 (PE tile T0), odd head
                        # on 64:128 (T8): alternating their 64-row S matmuls
                        # runs them concurrently on the two array halves.
                        o_ps = {}
                        for z in range(2):
                            o_ps[z] = psO.tile([65, 512], F32,
                                               name=f"o{j}{hp}{z}", tag="psO")
                        p_tiles = {0: [], 1: []}
                        for ib in range(0, n_i, 4):
                            ie = min(ib + 4, n_i)
                            for i in range(ib, ie):
                                for z in range(2):
                                    rows = slice(64 * z, 64 * z + 64)
                                    s_ps = psS.tile([128, 512], F32,
                                                    name=f"s{j}{hp}{i}{z}",
                                                    tag="psS")
                                    nc.tensor.matmul(
                                        s_ps[:],
                                        kT[hp][rows, 128 * i:128 * (i + 1)],
                                        qT[hp][rows, js], start=True, stop=True)
                                    p_sb = pp.tile([128, 512], F32R,
                                                   name=f"p{j}{hp}{i}{z}",
                                                   tag="pp")
                                    if i >= 4 * j:  # mixed tile: mask post-exp
                                        e_sb = ep.tile([128, 512], F32,
                                                       name=f"e{j}{hp}{i}{z}",
                                                       tag="ep")
                                        nc.scalar.activation(e_sb[:], s_ps[:],
                                                             AF.Exp, scale=SCALE)
                                        nc.vector.tensor_tensor(
                                            p_sb[:], e_sb[:],
                                            masks[i - 4 * j][:], op=ALU.mult)
                                    else:
                                        nc.scalar.activation(p_sb[:], s_ps[:],
                                                             AF.Exp, scale=SCALE)
                                    p_tiles[z].append(p_sb)
                            for i in range(ib, ie):
                                for z in range(2):
                                    nc.tensor.matmul(
                                        o_ps[z][:], vp[i][:, 2 * hp + z, :],
                                        p_tiles[z][i][:],
                                        start=(i == 0), stop=(i == n_i - 1))
                        # evacuate O' psum quickly (frees the bank), then
                        # normalize rows 0..63 by sums row 64 from SBUF
                        for z in range(2):
                            rows = slice(64 * z, 64 * z + 64)
                            ov_sb = ovp.tile([65, 512], F32,
                                             name=f"ov{j}{hp}{z}", tag="ovp")
                            nc.scalar.copy(ov_sb[:], o_ps[z][:])
                            r_sb = rp.tile([1, 512], F32, name=f"r{j}{hp}{z}",
                                           tag="rp")
                            nc.vector.reciprocal(r_sb[:], ov_sb[64:65, :])
                            bc_sb = bcp.tile([64, 512], F32,
                                             name=f"bc{j}{hp}{z}", tag="bcp")
                            nc.gpsimd.partition_broadcast(bc_sb[:], r_sb[:])
                            nc.vector.tensor_tensor(
                                OF[hp][rows, js], ov_sb[0:64, :], bc_sb[:],
                                op=ALU.mult)
                    # ---- phase 4 for the q-chunks completed by this j ----
                    for qc in range(4 * j, 4 * j + 4):
                        qs = slice(128 * qc, 128 * (qc + 1))
                        for n in range(2):
                            ns = slice(512 * n, 512 * (n + 1))
                            f_ps = psF.tile([128, 512], F32,
                                            name=f"f{qc}{n}", tag="psF")
                            for t in range(NCC):
                                nc.tensor.matmul(
                                    f_ps[:], OF[t][:, qs], wo_sb[t][:, ns],
                                    start=(t == 0), stop=(t == NCC - 1))
                            o_sb = osb.tile([128, 512], F32,
                                            name=f"ob{qc}{n}", tag="osb")
                            nc.scalar.copy(o_sb[:], f_ps[:])
                            nc.sync.dma_start(out[qs, ns], o_sb[:])

    nc.finalize()
    return nc


def _get_nc():
    if "nc" not in _CACHED:
        _CACHED["nc"] = build_nc()
    return _CACHED["nc"]


def kernel(x, W_q, W_k, W_v, W_out, trace=False, trace_kwargs=None):
    x = np.asarray(x, dtype=np.float32)
    W_q = np.asarray(W_q, dtype=np.float32)
    W_k = np.asarray(W_k, dtype=np.float32)
    W_v = np.asarray(W_v, dtype=np.float32)
    W_out = np.asarray(W_out, dtype=np.float32)

    nc = _get_nc()
    in_maps = []
    for core in range(8):
        b, g = core // 2, core % 2
        cs = slice(C * g, C * (g + 1))
        in_maps.append({
            "xT": np.ascontiguousarray(x[b].T),
            "wq": np.ascontiguousarray(W_q[:, cs]),
            "wk": np.ascontiguousarray(W_k[:, cs]),
            "wv": np.ascontiguousarray(W_v[:, cs]),
            "wo": np.ascontiguousarray(W_out[cs, :]),
        })
    res = run_bass_kernel_spmd(nc, in_maps, core_ids=list(range(8)),
                               trace=trace, **(trace_kwargs or {}))
    out = np.empty((B, L, D), dtype=np.float32)
    for b in range(B):
        out[b] = res.results[2 * b]["out"] + res.results[2 * b + 1]["out"]
        # q=0 is fully masked -> reference softmax gives uniform attention over
        # all of V; the device leaves NaN/0 in that row, patch it here.
        out[b, 0, :] = (x[b].mean(axis=0) @ W_v) @ W_out
    if trace:
        return out, res
    return out



# revision 8
# speedup vs baseline: 1.7741x; 1.7741x over previous
"""Multi-head causal attention (b=4, l=2048, d=1024, 16 heads x 64) on 8 trn2 cores.

Sharding: core c handles batch (c // 2) and head-group (c % 2) of 8 heads.
Each core computes a partial output x[b] @ W (its 8 heads' contribution);
the host sums the two partials per batch.

v2: all matmul operands bf16 (1 cyc/row guaranteed, FWL weight loads).
Exp batched per z-pair ([128,1024] psum groups). Causal suffix-trimming of
the diagonal S/exp/PV tiles. Masking via in-place gpsimd affine_select
(frees DVE). Normalization via reciprocal_approx_fast on the sums row +
partition_broadcast + one fused DVE multiply (no 3.3us iterative recip).
PSUM evacuations split between ACT (phase 1/2) and DVE (phase 3/4).

Device layouts (per core):
  xT      [1024, 2048] bf16  x[b] transposed on host (d on partitions)
  wq/wk   [1024, 512]  bf16  head-group column slices (natural layout, lhsT)
  wv      [1024, 512]  bf16
  wo      [512, 1024]  bf16  head-group row slice (rhs)
  qT/kT   [512, 2048]  bf16  c on partitions (4 sbuf tensors of 128)
  v_pad   16 x [128, 8, 65] bf16  v natural, per head 64 cols + ones col
  S^T     [128 m, 2x512 q] f32 psum pairs -> exp -> P^T bf16;
  PV: O'^T = [V|1]^T P^T gives sums row; softmax has no max-subtraction
  (scores are O(1)); the fully-masked q=0 column is fixed up on host.
"""

import sys

sys.path.insert(0, "/opt/trn_rl_repo")

import ml_dtypes
import numpy as np

import concourse.bacc as bacc
import concourse.mybir as mybir
import concourse.tile as tile
from concourse.bass_utils import run_bass_kernel_spmd

F32 = mybir.dt.float32
BF16 = mybir.dt.bfloat16
AF = mybir.ActivationFunctionType
ALU = mybir.AluOpType

B, L, D = 4, 2048, 1024
N_HEAD, KEY_DIM = 16, 64
HG = 8               # heads per core (head-group)
C = HG * KEY_DIM     # 512 per-core qkv width
SCALE = 1.0 / 8.0    # 1/sqrt(KEY_DIM)
NLC = 16             # l chunks of 128
NJ = 4               # q chunks of 512
ND = 8               # d chunks of 128
NCC = 4              # c chunks of 128

_CACHED = {}


def build_nc():
    nc = bacc.Bacc("TRN2", target_bir_lowering=False, debug=False)

    xT = nc.dram_tensor("xT", [D, L], BF16, kind="ExternalInput")
    wq = nc.dram_tensor("wq", [D, C], BF16, kind="ExternalInput")
    wk = nc.dram_tensor("wk", [D, C], BF16, kind="ExternalInput")
    wv = nc.dram_tensor("wv", [D, C], BF16, kind="ExternalInput")
    wo = nc.dram_tensor("wo", [C, D], BF16, kind="ExternalInput")
    out = nc.dram_tensor("out", [L, D], F32, kind="ExternalOutput")

    with tile.TileContext(nc) as tc:
        # ---- persistent pools (live across all phases) ----
        with tc.tile_pool(name="persist", bufs=1) as persist:

            qT = [persist.tile([128, L], BF16, name=f"qT{t}") for t in range(NCC)]
            # kT split per head with the other head's 64 rows zeroed: S matmuls
            # then use full-128-row stationaries (adjacent row-disjoint bf16
            # matmuls hw-fault; the zero rows annihilate the other head's q)
            kTp = [persist.tile([128, L], BF16, name=f"kTp{h}") for h in range(HG)]
            vp = [persist.tile([128, HG, KEY_DIM + 1], BF16, name=f"vp{i}")
                  for i in range(NLC)]

            for h in range(HG):
                nc.vector.memset(kTp[h][:], 0.0)
            for i in range(NLC):
                # whole-tile memset; phase-2 copies overwrite cols 0..63,
                # col 64 stays 1.0 (the sums column for PV)
                nc.vector.memset(vp[i][:], 1.0)

            # ---- phase 1+2: stream xT, project q/k/v ----
            with tc.tile_pool(name="wqkv", bufs=1) as wpool, \
                 tc.tile_pool(name="xt", bufs=16) as xtp, \
                 tc.tile_pool(name="psA", bufs=3, space="PSUM") as psA:
                wq_sb = [wpool.tile([128, C], BF16, name=f"wq{d}") for d in range(ND)]
                wk_sb = [wpool.tile([128, C], BF16, name=f"wk{d}") for d in range(ND)]
                wv_sb = [wpool.tile([128, C], BF16, name=f"wv{d}") for d in range(ND)]
                for d in range(ND):
                    nc.sync.dma_start(wq_sb[d][:], wq[128 * d:128 * (d + 1), :])
                    nc.sync.dma_start(wk_sb[d][:], wk[128 * d:128 * (d + 1), :])
                    nc.sync.dma_start(wv_sb[d][:], wv[128 * d:128 * (d + 1), :])

                for lc in range(NJ):  # 4 l-chunks of 512
                    ls = slice(512 * lc, 512 * (lc + 1))
                    xts = []
                    for d in range(ND):
                        t = xtp.tile([128, 512], BF16, name=f"xt{lc}_{d}", tag="xt")
                        nc.sync.dma_start(t[:], xT[128 * d:128 * (d + 1), ls])
                        xts.append(t)
                    # qT / kT chunks: out [128 c, 512 l]
                    for cc in range(NCC):
                        ps = psA.tile([128, 512], F32, name=f"pq{lc}{cc}", tag="psA")
                        for d in range(ND):
                            nc.tensor.matmul(
                                ps[:], wq_sb[d][:, 128 * cc:128 * (cc + 1)],
                                xts[d][:], start=(d == 0), stop=(d == ND - 1))
                        nc.scalar.copy(qT[cc][:, ls], ps[:])
                    for cc in range(NCC):
                        ps = psA.tile([128, 512], F32, name=f"pk{lc}{cc}", tag="psA")
                        for d in range(ND):
                            nc.tensor.matmul(
                                ps[:], wk_sb[d][:, 128 * cc:128 * (cc + 1)],
                                xts[d][:], start=(d == 0), stop=(d == ND - 1))
                        # split per head, keeping rows in place (z=1 stays on
                        # partitions 64..127); the other half remains zero
                        for z in range(2):
                            rows = slice(64 * z, 64 * z + 64)
                            nc.scalar.copy(kTp[2 * cc + z][rows, ls], ps[rows, :])
                    # v natural: out [128 l, 512 c] -> strided into vp
                    for lcc in range(4):
                        i = 4 * lc + lcc
                        ps = psA.tile([128, 512], F32, name=f"pv{i}", tag="psA")
                        for d in range(ND):
                            nc.tensor.matmul(
                                ps[:], xts[d][:, 128 * lcc:128 * (lcc + 1)],
                                wv_sb[d][:], start=(d == 0), stop=(d == ND - 1))
                        nc.scalar.copy(
                            vp[i][:, :, 0:KEY_DIM],
                            ps[:].rearrange("p (h c) -> p h c", h=HG))

            # ---- phase 3+4 (j-major): attention + output projection ----
            with tc.tile_pool(name="of", bufs=1) as ofp, \
                 tc.tile_pool(name="wo", bufs=1) as wop, \
                 tc.tile_pool(name="pp", bufs=5) as pp, \
                 tc.tile_pool(name="ovp", bufs=4) as ovp, \
                 tc.tile_pool(name="bcp", bufs=2) as bcp, \
                 tc.tile_pool(name="rp", bufs=2) as rp, \
                 tc.tile_pool(name="osb", bufs=3) as osb, \
                 tc.tile_pool(name="psS", bufs=2, space="PSUM") as psS, \
                 tc.tile_pool(name="psO", bufs=2, space="PSUM") as psO, \
                 tc.tile_pool(name="psF", bufs=2, space="PSUM") as psF:

                OF = [ofp.tile([128, L], BF16, name=f"of{t}") for t in range(NCC)]
                wo_sb = [wop.tile([128, D], BF16, name=f"wo{t}") for t in range(NCC)]
                for t in range(NCC):
                    nc.sync.dma_start(wo_sb[t][:], wo[128 * t:128 * (t + 1), :])

                for j in range(NJ):
                    js = slice(512 * j, 512 * (j + 1))
                    n_i = 4 * j + 4
                    for hp in range(4):  # head pairs share kT/qT tensor hp
                        o_ps = {}
                        for z in range(2):
                            o_ps[z] = psO.tile([65, 512], F32,
                                               name=f"o{j}{hp}{z}", tag="psO")

                        def emit_pv(item):
                            i, p_sb, st = item
                            for z in range(2):
                                nc.tensor.matmul(
                                    o_ps[z][:, st:512],
                                    vp[i][:, 2 * hp + z, :],
                                    p_sb[:, 512 * z + st:512 * z + 512],
                                    start=(i == 0), stop=(i == n_i - 1))

                        pend = []
                        for i in range(n_i):
                            r = i - 4 * j           # >=0: diagonal tile
                            st = 128 * r if r > 0 else 0
                            w = 512 - st
                            # S^T pair: [128 m, q] for both heads (z) of hp
                            sp = psS.tile([128, 1024], F32,
                                          name=f"s{j}{hp}{i}", tag="psS")
                            for z in range(2):
                                nc.tensor.matmul(
                                    sp[:, 512 * z + st:512 * z + 512],
                                    kTp[2 * hp + z][:, 128 * i:128 * (i + 1)],
                                    qT[hp][:, 512 * j + st:512 * j + 512],
                                    start=True, stop=True)
                            p_sb = pp.tile([128, 1024], BF16,
                                           name=f"p{j}{hp}{i}", tag="pp")
                            if r <= 0:
                                nc.scalar.activation(p_sb[:], sp[:],
                                                     AF.Exp, scale=SCALE)
                            else:
                                for z in range(2):
                                    cs = slice(512 * z + st, 512 * z + 512)
                                    nc.scalar.activation(p_sb[:, cs], sp[:, cs],
                                                         AF.Exp, scale=SCALE)
                            if r >= 0:  # mask: keep strictly q > m
                                for z in range(2):
                                    cs = slice(512 * z + st, 512 * z + 512)
                                    nc.gpsimd.affine_select(
                                        out=p_sb[:, cs], in_=p_sb[:, cs],
                                        compare_op=ALU.is_gt, fill=0.0,
                                        base=0, channel_multiplier=-1,
                                        pattern=[[1, w]])
                            pend.append((i, p_sb, st))
                            if len(pend) >= 2:
                                emit_pv(pend.pop(0))
                        for item in pend:
                            emit_pv(item)

                        # normalize: rows 0..63 by the sums row 64
                        for z in range(2):
                            ov = ovp.tile([65, 512], F32,
                                          name=f"ov{j}{hp}{z}", tag="ovp")
                            nc.vector.tensor_copy(ov[:], o_ps[z][:])
                            # recip_approx is a custom DVE op that ignores the
                            # input base partition -> stage the sums row to
                            # partition 0 with a standard copy first
                            srow = rp.tile([1, 512], F32, name=f"sr{j}{hp}{z}",
                                           tag="srp")
                            nc.vector.tensor_copy(srow[:], ov[64:65, :])
                            r_sb = rp.tile([1, 512], F32, name=f"r{j}{hp}{z}",
                                           tag="rp")
                            nc.vector.reciprocal_approx_fast(
                                r_sb[:], srow[:])
                            bc = bcp.tile([64, 512], F32,
                                          name=f"bc{j}{hp}{z}", tag="bcp")
                            nc.gpsimd.partition_broadcast(bc[:], r_sb[:])
                            nc.vector.tensor_tensor(
                                OF[hp][slice(64 * z, 64 * z + 64), js],
                                ov[0:64, :], bc[:], op=ALU.mult)

                    # ---- phase 4 for the q-chunks completed by this j ----
                    for qc in range(4 * j, 4 * j + 4):
                        qs = slice(128 * qc, 128 * (qc + 1))
                        for n in range(2):
                            ns = slice(512 * n, 512 * (n + 1))
                            f_ps = psF.tile([128, 512], F32,
                                            name=f"f{qc}{n}", tag="psF")
                            for t in range(NCC):
                                nc.tensor.matmul(
                                    f_ps[:], OF[t][:, qs], wo_sb[t][:, ns],
                                    start=(t == 0), stop=(t == NCC - 1))
                            o_sb = osb.tile([128, 512], F32,
                                            name=f"ob{qc}{n}", tag="osb")
                            nc.vector.tensor_copy(o_sb[:], f_ps[:])
                            nc.sync.dma_start(out[qs, ns], o_sb[:])

    nc.finalize()
    return nc


def _get_nc():
    if "nc" not in _CACHED:
        _CACHED["nc"] = build_nc()
    return _CACHED["nc"]


def _bf16(a):
    return np.ascontiguousarray(a).astype(ml_dtypes.bfloat16)


def kernel(x, W_q, W_k, W_v, W_out, trace=False, trace_kwargs=None):
    x = np.asarray(x, dtype=np.float32)
    W_q = np.asarray(W_q, dtype=np.float32)
    W_k = np.asarray(W_k, dtype=np.float32)
    W_v = np.asarray(W_v, dtype=np.float32)
    W_out = np.asarray(W_out, dtype=np.float32)

    nc = _get_nc()
    in_maps = []
    for core in range(8):
        b, g = core // 2, core % 2
        cs = slice(C * g, C * (g + 1))
        in_maps.append({
            "xT": _bf16(x[b].T),
            "wq": _bf16(W_q[:, cs]),
            "wk": _bf16(W_k[:, cs]),
            "wv": _bf16(W_v[:, cs]),
            "wo": _bf16(W_out[cs, :]),
        })
    res = run_bass_kernel_spmd(nc, in_maps, core_ids=list(range(8)),
                               trace=trace, **(trace_kwargs or {}))
    out = np.empty((B, L, D), dtype=np.float32)
    for b in range(B):
        out[b] = res.results[2 * b]["out"] + res.results[2 * b + 1]["out"]
        # q=0 is fully masked -> reference softmax gives uniform attention over
        # all of V; the device leaves NaN/garbage in that row, patch it here.
        out[b, 0, :] = (x[b].mean(axis=0) @ W_v) @ W_out
    if trace:
        return out, res
    return out


# revision 9
# speedup vs baseline: 2.0045x; 1.1299x over previous
"""Multi-head causal attention (b=4, l=2048, d=1024, 16 heads x 64) on 8 trn2 cores.

Sharding: core c handles batch (c // 2) and head-group (c % 2) of 8 heads.
Each core computes a partial output x[b] @ W (its 8 heads' contribution);
the host sums the two partials per batch.

v3: all matmul operands bf16. Software-pipelined phase structure: the
projection of l-chunk lc=j+1 and the output-projection of q-chunks of j-1
are interleaved into attention j's head-pair loop, filling the PE during
ACT(exp)-bound stretches. Causal suffix-trimming of diagonal tiles.
Masking via in-place gpsimd affine_select. Normalization via
reciprocal_approx_fast (sums row staged to partition 0 first - the custom
DVE op ignores the input base partition) + partition_broadcast + fused
DVE multiply.

Device layouts (per core):
  xT      [1024, 2048] bf16  x[b] transposed on host (d on partitions)
  wq/wk   [1024, 512]  bf16  head-group column slices (natural layout, lhsT)
  wv      [1024, 512]  bf16
  wo      [512, 1024]  bf16  head-group row slice (rhs)
  qT      [512, 2048]  bf16  c on partitions (4 sbuf tensors of 128)
  kTp     8 x [128, 2048] bf16  per-head, other head's 64 rows zeroed
          (adjacent row-disjoint bf16 matmuls hw-fault; zero rows
          annihilate the other head's q so stationaries are full 128-row)
  v_pad   16 x [128, 8, 65] bf16  v natural, per head 64 cols + ones col
  S^T     [128 m, 2x512 q] f32 psum pairs -> one exp -> P^T bf16
  PV: O'^T = [V|1]^T P^T gives sums row; softmax has no max-subtraction
  (scores are O(1)); the fully-masked q=0 column is fixed up on host.
"""

import sys

sys.path.insert(0, "/opt/trn_rl_repo")

import ml_dtypes
import numpy as np

import concourse.bacc as bacc
import concourse.mybir as mybir
import concourse.tile as tile
from concourse.bass_utils import run_bass_kernel_spmd

F32 = mybir.dt.float32
BF16 = mybir.dt.bfloat16
AF = mybir.ActivationFunctionType
ALU = mybir.AluOpType

B, L, D = 4, 2048, 1024
N_HEAD, KEY_DIM = 16, 64
HG = 8               # heads per core (head-group)
C = HG * KEY_DIM     # 512 per-core qkv width
SCALE = 1.0 / 8.0    # 1/sqrt(KEY_DIM)
NLC = 16             # l chunks of 128
NJ = 4               # q chunks of 512
ND = 8               # d chunks of 128
NCC = 4              # c chunks of 128

_CACHED = {}


def build_nc():
    nc = bacc.Bacc("TRN2", target_bir_lowering=False, debug=False)

    xT = nc.dram_tensor("xT", [D, L], BF16, kind="ExternalInput")
    wq = nc.dram_tensor("wq", [D, C], BF16, kind="ExternalInput")
    wk = nc.dram_tensor("wk", [D, C], BF16, kind="ExternalInput")
    wv = nc.dram_tensor("wv", [D, C], BF16, kind="ExternalInput")
    wo = nc.dram_tensor("wo", [C, D], BF16, kind="ExternalInput")
    out = nc.dram_tensor("out", [L, D], F32, kind="ExternalOutput")

    with tile.TileContext(nc) as tc:
        with tc.tile_pool(name="persist", bufs=1) as persist:
            qT = [persist.tile([128, L], BF16, name=f"qT{t}") for t in range(NCC)]
            kTp = [persist.tile([128, L], BF16, name=f"kTp{h}") for h in range(HG)]
            vp = [persist.tile([128, HG, KEY_DIM + 1], BF16, name=f"vp{i}")
                  for i in range(NLC)]
            OF = [persist.tile([128, L], BF16, name=f"of{t}") for t in range(NCC)]
            wo_sb = [persist.tile([128, D], BF16, name=f"wo{t}") for t in range(NCC)]

            for h in range(HG):
                nc.vector.memset(kTp[h][:], 0.0)
            for i in range(NLC):
                # col 64 stays 1.0 (the PV sums column)
                nc.vector.memset(vp[i][:], 1.0)

            with tc.tile_pool(name="wqkv", bufs=1) as wpool, \
                 tc.tile_pool(name="xt", bufs=16) as xtp, \
                 tc.tile_pool(name="pp", bufs=5) as pp, \
                 tc.tile_pool(name="ovp", bufs=4) as ovp, \
                 tc.tile_pool(name="bcp", bufs=2) as bcp, \
                 tc.tile_pool(name="rp", bufs=2) as rp, \
                 tc.tile_pool(name="osb", bufs=3) as osb, \
                 tc.tile_pool(name="psA", bufs=2, space="PSUM") as psA, \
                 tc.tile_pool(name="psS", bufs=2, space="PSUM") as psS, \
                 tc.tile_pool(name="psO", bufs=2, space="PSUM") as psO:

                wq_sb = [wpool.tile([128, C], BF16, name=f"wq{d}") for d in range(ND)]
                wk_sb = [wpool.tile([128, C], BF16, name=f"wk{d}") for d in range(ND)]
                wv_sb = [wpool.tile([128, C], BF16, name=f"wv{d}") for d in range(ND)]

                xts_all = {}

                def prefetch_xts(lc):
                    ts = []
                    ls = slice(512 * lc, 512 * (lc + 1))
                    for d in range(ND):
                        t = xtp.tile([128, 512], BF16, name=f"xt{lc}_{d}", tag="xt")
                        nc.sync.dma_start(t[:], xT[128 * d:128 * (d + 1), ls])
                        ts.append(t)
                    xts_all[lc] = ts

                # startup DMA order: wq, x(lc=0), wk, wv, wo
                for d in range(ND):
                    nc.sync.dma_start(wq_sb[d][:], wq[128 * d:128 * (d + 1), :])
                prefetch_xts(0)
                for d in range(ND):
                    nc.sync.dma_start(wk_sb[d][:], wk[128 * d:128 * (d + 1), :])
                for d in range(ND):
                    nc.sync.dma_start(wv_sb[d][:], wv[128 * d:128 * (d + 1), :])
                for t in range(NCC):
                    nc.sync.dma_start(wo_sb[t][:], wo[128 * t:128 * (t + 1), :])

                def proj_chunk(lc, part):
                    """1/4 of the q/k/v projection for l-chunk lc."""
                    ls = slice(512 * lc, 512 * (lc + 1))
                    xts = xts_all[lc]
                    cc = part
                    # q chunk
                    ps = psA.tile([128, 512], F32, name=f"pq{lc}{cc}", tag="psA")
                    for d in range(ND):
                        nc.tensor.matmul(
                            ps[:], wq_sb[d][:, 128 * cc:128 * (cc + 1)],
                            xts[d][:], start=(d == 0), stop=(d == ND - 1))
                    nc.scalar.copy(qT[cc][:, ls], ps[:])
                    # k chunk, split per head with rows kept in place
                    ps = psA.tile([128, 512], F32, name=f"pk{lc}{cc}", tag="psA")
                    for d in range(ND):
                        nc.tensor.matmul(
                            ps[:], wk_sb[d][:, 128 * cc:128 * (cc + 1)],
                            xts[d][:], start=(d == 0), stop=(d == ND - 1))
                    for z in range(2):
                        rows = slice(64 * z, 64 * z + 64)
                        nc.vector.tensor_copy(kTp[2 * cc + z][rows, ls],
                                              ps[rows, :])
                    # v chunk (l natural)
                    lcc = part
                    i = 4 * lc + lcc
                    ps = psA.tile([128, 512], F32, name=f"pv{i}", tag="psA")
                    for d in range(ND):
                        nc.tensor.matmul(
                            ps[:], xts[d][:, 128 * lcc:128 * (lcc + 1)],
                            wv_sb[d][:], start=(d == 0), stop=(d == ND - 1))
                    nc.scalar.copy(
                        vp[i][:, :, 0:KEY_DIM],
                        ps[:].rearrange("p (h c) -> p h c", h=HG))

                def phase4_chunk(j, qc):
                    """Output projection for one 128-row q chunk."""
                    qs = slice(128 * qc, 128 * (qc + 1))
                    for n in range(2):
                        ns = slice(512 * n, 512 * (n + 1))
                        f_ps = psA.tile([128, 512], F32,
                                        name=f"f{qc}{n}", tag="psA")
                        for t in range(NCC):
                            nc.tensor.matmul(
                                f_ps[:], OF[t][:, qs], wo_sb[t][:, ns],
                                start=(t == 0), stop=(t == NCC - 1))
                        o_sb = osb.tile([128, 512], F32,
                                        name=f"ob{qc}{n}", tag="osb")
                        nc.vector.tensor_copy(o_sb[:], f_ps[:])
                        nc.sync.dma_start(out[qs, ns], o_sb[:])

                def attn_hp(j, hp):
                    js = slice(512 * j, 512 * (j + 1))
                    n_i = 4 * j + 4
                    o_ps = {}
                    for z in range(2):
                        o_ps[z] = psO.tile([65, 512], F32,
                                           name=f"o{j}{hp}{z}", tag="psO")

                    def emit_pv(item):
                        i, p_sb, st = item
                        for z in range(2):
                            nc.tensor.matmul(
                                o_ps[z][:, st:512],
                                vp[i][:, 2 * hp + z, :],
                                p_sb[:, 512 * z + st:512 * z + 512],
                                start=(i == 0), stop=(i == n_i - 1))

                    pend = []
                    for i in range(n_i):
                        r = i - 4 * j           # >=0: diagonal tile
                        st = 128 * r if r > 0 else 0
                        w = 512 - st
                        sp = psS.tile([128, 1024], F32,
                                      name=f"s{j}{hp}{i}", tag="psS")
                        for z in range(2):
                            nc.tensor.matmul(
                                sp[:, 512 * z + st:512 * z + 512],
                                kTp[2 * hp + z][:, 128 * i:128 * (i + 1)],
                                qT[hp][:, 512 * j + st:512 * j + 512],
                                start=True, stop=True)
                        p_sb = pp.tile([128, 1024], BF16,
                                       name=f"p{j}{hp}{i}", tag="pp")
                        if r <= 0:
                            nc.scalar.activation(p_sb[:], sp[:],
                                                 AF.Exp, scale=SCALE)
                        else:
                            for z in range(2):
                                cs = slice(512 * z + st, 512 * z + 512)
                                nc.scalar.activation(p_sb[:, cs], sp[:, cs],
                                                     AF.Exp, scale=SCALE)
                        if r >= 0:  # mask: keep strictly q > m
                            for z in range(2):
                                cs = slice(512 * z + st, 512 * z + 512)
                                nc.gpsimd.affine_select(
                                    out=p_sb[:, cs], in_=p_sb[:, cs],
                                    compare_op=ALU.is_gt, fill=0.0,
                                    base=0, channel_multiplier=-1,
                                    pattern=[[1, w]])
                        pend.append((i, p_sb, st))
                        if len(pend) >= 2:
                            emit_pv(pend.pop(0))
                    for item in pend:
                        emit_pv(item)

                    # normalize rows 0..63 by the sums row 64
                    for z in range(2):
                        ov = ovp.tile([65, 512], F32,
                                      name=f"ov{j}{hp}{z}", tag="ovp")
                        nc.vector.tensor_copy(ov[:], o_ps[z][:])
                        srow = rp.tile([1, 512], F32, name=f"sr{j}{hp}{z}",
                                       tag="srp")
                        nc.vector.tensor_copy(srow[:], ov[64:65, :])
                        r_sb = rp.tile([1, 512], F32, name=f"r{j}{hp}{z}",
                                       tag="rp")
                        nc.vector.reciprocal_approx_fast(r_sb[:], srow[:])
                        bc = bcp.tile([64, 512], F32,
                                      name=f"bc{j}{hp}{z}", tag="bcp")
                        nc.gpsimd.partition_broadcast(bc[:], r_sb[:])
                        nc.vector.tensor_tensor(
                            OF[hp][slice(64 * z, 64 * z + 64), js],
                            ov[0:64, :], bc[:], op=ALU.mult)

                # ---- pipelined schedule ----
                for part in range(4):
                    proj_chunk(0, part)
                prefetch_xts(1)
                for j in range(NJ):
                    if j + 2 <= 3:
                        prefetch_xts(j + 2)
                    for hp in range(4):
                        attn_hp(j, hp)
                        if j < 3:
                            proj_chunk(j + 1, hp)
                        if j > 0:
                            phase4_chunk(j - 1, 4 * (j - 1) + hp)
                for hp in range(4):
                    phase4_chunk(3, 12 + hp)

    nc.finalize()
    return nc


def _get_nc():
    if "nc" not in _CACHED:
        _CACHED["nc"] = build_nc()
    return _CACHED["nc"]


def _bf16(a):
    return np.ascontiguousarray(a).astype(ml_dtypes.bfloat16)


def kernel(x, W_q, W_k, W_v, W_out, trace=False, trace_kwargs=None):
    x = np.asarray(x, dtype=np.float32)
    W_q = np.asarray(W_q, dtype=np.float32)
    W_k = np.asarray(W_k, dtype=np.float32)
    W_v = np.asarray(W_v, dtype=np.float32)
    W_out = np.asarray(W_out, dtype=np.float32)

    nc = _get_nc()
    in_maps = []
    for core in range(8):
        b, g = core // 2, core % 2
        cs = slice(C * g, C * (g + 1))
        in_maps.append({
            "xT": _bf16(x[b].T),
            "wq": _bf16(W_q[:, cs]),
            "wk": _bf16(W_k[:, cs]),
            "wv": _bf16(W_v[:, cs]),
            "wo": _bf16(W_out[cs, :]),
        })
    res = run_bass_kernel_spmd(nc, in_maps, core_ids=list(range(8)),
                               trace=trace, **(trace_kwargs or {}))
    out = np.empty((B, L, D), dtype=np.float32)
    for b in range(B):
        out[b] = res.results[2 * b]["out"] + res.results[2 * b + 1]["out"]
        # q=0 is fully masked -> reference softmax gives uniform attention
        # over all of V; the device leaves garbage in that row, patch here.
        out[b, 0, :] = (x[b].mean(axis=0) @ W_v) @ W_out
    if trace:
        return out, res
    return out
